# revision 22
# baseline (speedup 1.0000x reference)
"""HGT link predictor on 8 trn2 NeuronCores — v2.

Sharding: nodes split 8 ways per type with host-side load balancing so every
(core, 128-dst-window) holds <= KCH*128 edges per relation (KCH=3 typical).
Params replicated. Per layer: raw k/v projections for source types (0,1) in
bf16 -> per-type Shared-output AllGather (overlapped with other projections /
previous edge phases) -> per-relation q projections with a_rel folded in
(logits = q_r . k_raw) -> per-edge dma_gather of k|v and q_r (bf16) ->
logits via multiply+segmented reduce -> exp -> scatter-add via one-hot
matmuls into PSUM (softmax denominator rides as columns 256..263) ->
normalize -> per-relation m_rel block-diag post-transform -> gelu -> output
linear (skip gate folded) -> residual+LayerNorm+relu. All matmuls bf16 with
fp32 PSUM accumulation; LayerNorm statistics in fp32.
"""
import math
import numpy as np

import concourse.bacc as bacc
import concourse.bass as bass
import concourse.mybir as mybir
import concourse.tile as tile
from concourse.bass_utils import run_bass_kernel_spmd
from concourse.library_config import mlp

F32 = mybir.dt.float32
BF16 = mybir.dt.bfloat16
BF16_NP = mybir.dt.np(mybir.dt.bfloat16)
I16 = mybir.dt.int16
AF = mybir.ActivationFunctionType
OP = mybir.AluOpType

T, R, L = 3, 4, 2
H, HEADS, D, FIN, OUT = 256, 8, 32, 128, 128
SRC_T = (0, 1, 1, 1)
DST_T = (1, 0, 1, 2)
LN_EPS = 1e-5
NC = 8
N = 20000
NL = N // NC          # 2500 real local nodes per type
NT = 20               # node tiles of 128
NLP = NT * 128        # 2560 padded local nodes
NWIN = NT             # dst windows of 128 local slots
NBINS = NC * NWIN     # 160
CAP = N // NBINS      # 125 real nodes per window
GWIN = 4              # windows per gather group


def _block_diag(a):
    """a: [HEADS, D, D] -> [H, H] block diagonal."""
    out = np.zeros((H, H), np.float32)
    for h in range(HEADS):
        out[h * D:(h + 1) * D, h * D:(h + 1) * D] = a[h]
    return out


def _wrap_idx(idx):
    """idx [M] -> [128, M//16] int16 wrapped in 16 partitions, replicated."""
    m = idx.shape[0]
    assert m % 16 == 0
    w = np.zeros((128, m // 16), np.int16)
    w[:16] = idx.astype(np.int16).reshape(m // 16, 16).T
    for rep in range(1, 8):
        w[16 * rep:16 * rep + 16] = w[:16]
    return w


def _balance(deg_dims):
    """deg_dims: [K, N] per-relation degrees. Greedy best-fit-decreasing into
    NBINS bins of CAP nodes, minimizing the max per-dimension bin load.
    Returns core_of [N], slot_of [N]."""
    n = deg_dims.shape[1]
    order = np.argsort(-deg_dims.sum(0), kind="stable")
    loads = np.zeros((deg_dims.shape[0], NBINS), np.float64)
    counts = np.zeros(NBINS, np.int64)
    binof = np.empty(n, np.int64)
    for node in order:
        cand = (loads + deg_dims[:, node:node + 1]).max(0) + 1e-6 * loads.sum(0)
        cand[counts >= CAP] = np.inf
        b = int(np.argmin(cand))
        binof[node] = b
        loads[:, b] += deg_dims[:, node]
        counts[b] += 1
    core_of = binof // NWIN
    win = binof % NWIN
    # slot within window: dense by order of node id
    slot_of = np.empty(n, np.int64)
    for b in range(NBINS):
        nodes = np.where(binof == b)[0]
        w = b % NWIN
        slot_of[nodes] = 128 * w + np.arange(len(nodes))
    return core_of, slot_of


def _preprocess(inputs):
    x = np.asarray(inputs["x"], np.float32)
    edge_index = np.asarray(inputs["edge_index"])
    Win = np.asarray(inputs["Win"], np.float32)
    b_in = np.asarray(inputs["b_in"], np.float32)
    Wk = np.asarray(inputs["Wk"], np.float32); bk = np.asarray(inputs["bk"], np.float32)
    Wq = np.asarray(inputs["Wq"], np.float32); bq = np.asarray(inputs["bq"], np.float32)
    Wv = np.asarray(inputs["Wv"], np.float32); bv = np.asarray(inputs["bv"], np.float32)
    Wa = np.asarray(inputs["Wa"], np.float32); ba = np.asarray(inputs["ba"], np.float32)
    skip = np.asarray(inputs["skip"], np.float32)
    a_rel = np.asarray(inputs["a_rel"], np.float32)
    m_rel = np.asarray(inputs["m_rel"], np.float32)
    p_rel = np.asarray(inputs["p_rel"], np.float32)
    ln_g = np.asarray(inputs["ln_g"], np.float32)
    ln_b = np.asarray(inputs["ln_b"], np.float32)
    Wout = np.asarray(inputs["Wout"], np.float32)
    bout = np.asarray(inputs["bout"], np.float32)

    meta = {}
    inv_sqrt_d = 1.0 / math.sqrt(D)

    # folded weights ------------------------------------------------------
    # q-side: logits = (q @ MA) . k_raw with MA = blockdiag(a_h^T * p_h/sqrt(D))
    # v-side: v_r = v @ blockdiag(m_h) folded into Wv per relation
    wqr = np.zeros((L, R, H, H), np.float32); bqr = np.zeros((L, R, H), np.float32)
    wvr = np.zeros((L, R, H, H), np.float32); bvr = np.zeros((L, R, H), np.float32)
    for l in range(L):
        for r in range(R):
            dt, st = DST_T[r], SRC_T[r]
            ma = _block_diag(np.transpose(a_rel[l, r], (0, 2, 1))
                             * (p_rel[l, r] * inv_sqrt_d)[:, None, None])
            wqr[l, r] = Wq[l, dt] @ ma
            bqr[l, r] = bq[l, dt] @ ma
            mv = _block_diag(m_rel[l, r])
            wvr[l, r] = Wv[l, st] @ mv
            bvr[l, r] = bv[l, st] @ mv
    beta = 1.0 / (1.0 + np.exp(-skip))          # [L, T]
    g = beta / (2.0 - beta)
    wa_eff = Wa * g[:, :, None, None]
    ba_eff = ba * g[:, :, None]
    meta["eps_eff"] = (LN_EPS / (2.0 - beta) ** 2).tolist()

    meta["use_bias"] = dict(
        bin_=bool(np.any(b_in)), bqr=bool(np.any(bqr)),
        bk=bool(np.any(bk)) or bool(np.any(bvr)),
        ba=bool(np.any(ba_eff)), bout=bool(np.any(bout)),
        lng=not np.allclose(ln_g, 1.0), lnb=bool(np.any(ln_b)),
    )

    def bcast(v):
        # [..., F] -> [..., 128, F]: per-feature vectors replicated across partitions
        return np.ascontiguousarray(
            np.broadcast_to(v[..., None, :], v.shape[:-1] + (128, v.shape[-1])))

    # node balancing ------------------------------------------------------
    # per dst type, balance windows across the relations that aggregate there
    deg = np.zeros((R, N), np.int64)
    for r in range(R):
        np.add.at(deg[r], edge_index[r, 1], 1)
    rels_of_t = [[r for r in range(R) if DST_T[r] == t] for t in range(T)]
    core_of = np.zeros((T, N), np.int64)
    slot_of = np.zeros((T, N), np.int64)
    for t in range(T):
        core_of[t], slot_of[t] = _balance(deg[rels_of_t[t]])
    meta["core_of"] = core_of
    meta["slot_of"] = slot_of

    # edge partitioning ---------------------------------------------------
    win_edges = [[None] * R for _ in range(NC)]   # [c][r] -> list of (src, dslot) per win
    kch_need = 1
    for c in range(NC):
        for r in range(R):
            st, dt = SRC_T[r], DST_T[r]
            src = edge_index[r, 0].astype(np.int64)
            dst = edge_index[r, 1].astype(np.int64)
            sel = core_of[dt][dst] == c
            s, d = src[sel], slot_of[dt][dst[sel]]
            o = np.argsort(d, kind="stable")
            s, d = s[o], d[o]
            wins = []
            for w in range(NWIN):
                m = (d // 128) == w
                sw, dw = s[m], d[m]
                kch_need = max(kch_need, (len(sw) + 127) // 128)
                wins.append((sw, dw))
            win_edges[c][r] = wins
    KCH = kch_need
    meta["KCH"] = KCH
    NCHUNK = NWIN * KCH
    NIDX_R = NCHUNK * 128

    xT = np.ascontiguousarray(x.transpose(0, 2, 1))  # [T, FIN, N]
    per_core = []
    for c in range(NC):
        oh = np.zeros((R, NCHUNK, 128, 128), np.float32)
        kv_idx = np.zeros((R, NIDX_R), np.int64)
        qi_idx = np.zeros((R, NIDX_R), np.int64)
        for r in range(R):
            st = SRC_T[r]
            for w in range(NWIN):
                sw, dw = win_edges[c][r][w]
                ne = len(sw)
                base = w * KCH * 128
                kv_idx[r, base:base + ne] = core_of[st][sw] * NLP + slot_of[st][sw]
                qi_idx[r, base:base + ne] = dw
                ch = base // 128 + np.arange(ne) // 128
                oh[r, ch, np.arange(ne) % 128, dw - w * 128] = 1.0
        # partition-major one-hot: [R, 128(edge), NCHUNK, 128(col)]
        oh_pm = np.ascontiguousarray(oh.transpose(0, 2, 1, 3))
        xc = np.zeros((T, FIN, NLP), np.float32)
        for t in range(T):
            idx = np.where(core_of[t] == c)[0]
            xc[t][:, slot_of[t][idx]] = xT[t][:, idx]
        per_core.append(dict(
            xT_h=xc.astype(BF16_NP),
            oh=oh_pm.astype(BF16_NP),
            kv_idx=np.stack([_wrap_idx(kv_idx[r]) for r in range(R)]),
            qi_idx=np.stack([_wrap_idx(qi_idx[r]) for r in range(R)]),
        ))

    def b16(a):
        return np.ascontiguousarray(a).astype(BF16_NP)

    shared = dict(
        win=b16(Win),                                      # [3,128,256]
        wk=b16(Wk), wvr=b16(wvr),                          # [L,T,...]/[L,R,...]
        wqr=b16(wqr), wa=b16(wa_eff),
        wout=b16(Wout),
        ident=np.eye(128, dtype=np.float32).astype(BF16_NP),
        bin_b=bcast(b_in), bqr_b=bcast(bqr),
        bk_b=bcast(bk), bvr_b=bcast(bvr),
        ba_b=bcast(ba_eff), bout_b=bcast(bout),
        lng_b=bcast(ln_g), lnb_b=bcast(ln_b),
    )
    return shared, per_core, meta


def NIDX16(KCH):
    return NWIN * KCH * 128 // 16


def _build(nc, meta, shapes):
    KCH = meta["KCH"]
    NCHUNK = NWIN * KCH
    GC = GWIN * KCH                      # chunks per gather group
    NGRP = NWIN // GWIN
    ub = meta["use_bias"]
    eps_eff = meta["eps_eff"]

    def din(name, dtype=BF16):
        if "idx" in name:
            dtype = I16
        return nc.dram_tensor(name, shapes[name], dtype, kind="ExternalInput").ap()

    xT_h = din("xT_h"); oh_d = din("oh")
    kv_idx_d = din("kv_idx"); qi_idx_d = din("qi_idx")
    win_d = din("win"); wk_d = din("wk"); wvr_d = din("wvr")
    wqr_d = din("wqr"); wa_d = din("wa")
    wout_d = din("wout")
    ident_d = din("ident")
    bias_d = {k: din(k, F32) for k in
              ("bin_b", "bqr_b", "bk_b", "bvr_b", "ba_b", "bout_b", "lng_b", "lnb_b")}
    y_d = nc.dram_tensor("y", [T, NLP, OUT], F32, kind="ExternalOutput").ap()

    def bc32(ap2d):
        """[..., k] AP -> [..., k, 32] stride-0 broadcast AP."""
        return bass.AP(tensor=ap2d.tensor, offset=ap2d.offset,
                       ap=list(ap2d.ap) + [[0, D]])

    with tile.TileContext(nc) as tc:
        with (
            tc.tile_pool(name="persist", bufs=1) as pp,
            tc.tile_pool(name="hTp", bufs=2) as hTp,
            tc.tile_pool(name="wpool", bufs=6) as wp,
            tc.tile_pool(name="stage", bufs=4) as stg,
            tc.tile_pool(name="edge", bufs=2) as ep,
            tc.tile_pool(name="edge1", bufs=2) as ep1,
            tc.tile_pool(name="small", bufs=4) as sp,
            tc.tile_pool(name="psB", bufs=2, space="PSUM") as psB,
            tc.tile_pool(name="psC", bufs=2, space="PSUM") as psC,
            tc.tile_pool(name="psD", bufs=4, space="PSUM") as psD,
            tc.tile_pool(name="dram", bufs=1, space="DRAM") as dp,
        ):
            nc.gpsimd.load_library(mlp)

            ident = pp.tile([128, 128], BF16, tag="ident")
            nc.sync.dma_start(ident[:], ident_d)
            h = pp.tile([128, T, NT, H], BF16, tag="h")
            agg1 = pp.tile([128, NT, H], BF16, tag="agg1")
            # edge indices: identical across layers, load once
            kvi = pp.tile([128, R, NIDX16(KCH)], I16, tag="kvi")
            qii = pp.tile([128, R, NIDX16(KCH)], I16, tag="qii")
            for r in range(R):
                nc.sync.dma_start(kvi[:, r, :], kv_idx_d[r])
                nc.sync.dma_start(qii[:, r, :], qi_idx_d[r])

            # per-layer double-buffered kv/q tables (no cross-layer WAR).
            # kv0 = [k0|v_r0] (512 cols, single per-edge gather); kv1 =
            # [k1|v_r1|v_r2|v_r3] (1024 cols): r1 gathers [k|v_r1] contiguously,
            # r2/r3 gather k and v_r separately.
            KVW = [2 * H, 4 * H]
            kv_loc = [[dp.tile([NLP, KVW[t]], BF16, name=f"kv_loc_{l}_{t}")
                       for t in range(2)] for l in range(L)]
            kv_full = [[dp.tile([NC * NLP, KVW[t]], BF16,
                                name=f"kv_full_{l}_{t}")
                        for t in range(2)] for l in range(L)]
            qr_dram = [[dp.tile([NLP, H], BF16, name=f"qr_{l}_{r}")
                        for r in range(R)] for l in range(L)]

            def load_w(src_ap):
                """[256, M] dram -> [128, 2, M] sbuf tile."""
                m = src_ap.shape[-1]
                t_ = wp.tile([128, 2, m], BF16, tag="w")
                nc.sync.dma_start(t_[:], src_ap.rearrange("(kt kp) m -> kp kt m", kp=128))
                return t_

            def load_bias(src_ap):
                t_ = wp.tile([128, H], F32, tag="bias")
                nc.sync.dma_start(t_[:], src_ap)
                return t_

            # ---- input projection: h[t] = relu(xT^T @ Win + b) ----
            def input_proj(t):
                w_in = wp.tile([128, H], BF16, tag="w")
                nc.sync.dma_start(w_in[:], win_d[t])
                bt = load_bias(bias_d["bin_b"][t]) if ub["bin_"] else None
                for nt in range(NT):
                    xt = wp.tile([128, 128], BF16, tag="xt")
                    nc.sync.dma_start(xt[:], xT_h[t, :, nt * 128:(nt + 1) * 128])
                    ps = psD.tile([128, H], F32, tag="pp")
                    nc.tensor.matmul(ps[:], xt[:], w_in[:], start=True, stop=True)
                    if bt is not None:
                        nc.vector.tensor_add(ps[:], ps[:], bt[:])
                    nc.scalar.activation(h[:, t, nt, :], ps[:], AF.Relu)

            def transposes(t):
                """h[:, t] -> fresh feature-major tile [128, 2, NT, 128]."""
                hT = hTp.tile([128, 2, NT, 128], BF16, tag="hT")
                for nt in range(NT):
                    for ft in range(2):
                        tp = psB.tile([128, 128], BF16)
                        nc.tensor.transpose(
                            tp[:], h[:, t, nt, ft * 128:(ft + 1) * 128], ident[:])
                        if (nt + ft) % 2:
                            nc.vector.tensor_copy(hT[:, ft, nt, :], tp[:])
                        else:
                            nc.scalar.copy(hT[:, ft, nt, :], tp[:])
                return hT

            def kv_proj_ag(l, t, hT):
                """k + per-relation v_r projections for src type t -> AllGather."""
                rels = [r for r in range(R) if SRC_T[r] == t]
                wk_t = load_w(wk_d[l, t])
                bk_t = load_bias(bias_d["bk_b"][l, t]) if ub["bk"] else None
                wv_ts, bv_ts = [], []
                for r in rels:
                    wv_ts.append(load_w(wvr_d[l, r]))
                    bv_ts.append(load_bias(bias_d["bvr_b"][l, r]) if ub["bk"] else None)
                ncols = KVW[t]
                for nt in range(NT):
                    st_ = stg.tile([128, 4 * H], BF16, tag="kvstage")
                    cols = [(wk_t, bk_t, 0)] + [
                        (wv_ts[i], bv_ts[i], (i + 1) * H) for i in range(len(rels))]
                    for j, (w_t, b_t, c0) in enumerate(cols):
                        ps = psD.tile([128, H], F32, tag="pp")
                        for kt in range(2):
                            nc.tensor.matmul(ps[:], hT[:, kt, nt, :], w_t[:, kt, :],
                                             start=(kt == 0), stop=(kt == 1))
                        if b_t is not None:
                            nc.vector.tensor_add(ps[:], ps[:], b_t[:])
                        if j % 2:
                            nc.vector.tensor_copy(st_[:, c0:c0 + H], ps[:])
                        else:
                            nc.scalar.copy(st_[:, c0:c0 + H], ps[:])
                    nc.sync.dma_start(
                        kv_loc[l][t][nt * 128:(nt + 1) * 128, :], st_[:, :ncols])
                nc.gpsimd.collective_compute(
                    "AllGather", OP.bypass,
                    replica_groups=[list(range(NC))],
                    ins=[kv_loc[l][t][:].opt()], outs=[kv_full[l][t][:].opt()],
                )

            def q_proj(l, r, hT):
                """folded q projection for relation r -> qr_dram[l][r]."""
                wq_t = load_w(wqr_d[l, r])
                bq_t = load_bias(bias_d["bqr_b"][l, r]) if ub["bqr"] else None
                for nt in range(NT):
                    ps = psD.tile([128, H], F32, tag="pp")
                    for kt in range(2):
                        nc.tensor.matmul(ps[:], hT[:, kt, nt, :], wq_t[:, kt, :],
                                         start=(kt == 0), stop=(kt == 1))
                    st_ = stg.tile([128, H], BF16, tag="qstage")
                    if bq_t is not None:
                        nc.vector.tensor_add(ps[:], ps[:], bq_t[:])
                        nc.scalar.copy(st_[:], ps[:])
                    else:
                        nc.scalar.copy(st_[:], ps[:])
                    nc.sync.dma_start(
                        qr_dram[l][r][nt * 128:(nt + 1) * 128, :], st_[:])

            s1 = pp.tile([128, T, NT], F32, tag="s1")
            s2 = pp.tile([128, T, NT], F32, tag="s2")
            sqs = pp.tile([128, H], F32, tag="sqs")

            def transpose2(dst3, src, lbl):
                """src [128, 256] (sbuf) -> dst3 [128, 2, 128] bf16."""
                for ft in range(2):
                    tp = psB.tile([128, 128], BF16)
                    nc.tensor.transpose(tp[:], src[:, ft * 128:(ft + 1) * 128], ident[:])
                    if (lbl + ft) % 2:
                        nc.vector.tensor_copy(dst3[:, ft, :], tp[:])
                    else:
                        nc.scalar.copy(dst3[:, ft, :], tp[:])

            def edge_setup(l, r):
                dt = DST_T[r]
                wa_t = load_w(wa_d[l, dt]) if r != 0 else None
                ba_t = (load_bias(bias_d["ba_b"][l, dt])
                        if r != 0 and ub["ba"] else None)
                return wa_t, ba_t

            def edge_group(l, r, gidx, ctx, par):
                st_t, dt = SRC_T[r], DST_T[r]
                wa_t, ba_t = ctx
                ni = GC * 128
                i0 = gidx * (GC * 8)          # idx col offset (int16 cols = idx/16)
                isl = kvi[:, r, i0:i0 + GC * 8]
                kvt = kv_full[l][st_t]
                # v_r column offset: kv0=[k0|v_r0]; kv1=[k1|v_r1|v_r2|v_r3]
                vc0 = H if r == 0 else r * H
                if r <= 1:
                    # k and v_r are adjacent: one 512-col gather
                    kvg = ep.tile([128, GC, 2 * H], BF16, tag="kvg")
                    nc.gpsimd.dma_gather(
                        kvg[:], kvt[:, 0:2 * H], isl,
                        ni, ni, 2 * H, elem_step=KVW[st_t], queue_num=par)
                    kg_ap, vg_ap = kvg[:, :, 0:H], kvg[:, :, H:2 * H]
                else:
                    kg = ep.tile([128, GC, H], BF16, tag="kg")
                    vg = ep.tile([128, GC, H], BF16, tag="vg")
                    nc.gpsimd.dma_gather(
                        kg[:], kvt[:, 0:H], isl,
                        ni, ni, H, elem_step=KVW[st_t], queue_num=par)
                    nc.gpsimd.dma_gather(
                        vg[:], kvt[:, vc0:vc0 + H], isl,
                        ni, ni, H, elem_step=KVW[st_t], queue_num=par)
                    kg_ap, vg_ap = kg[:], vg[:]
                qig = ep.tile([128, GC, H], BF16, tag="qig")
                nc.gpsimd.dma_gather(
                    qig[:], qr_dram[l][r][:], qii[:, r, i0:i0 + GC * 8],
                    ni, ni, H, queue_num=par)
                ohg = ep.tile([128, GC, 128], BF16, tag="ohg")
                nc.sync.dma_start(ohg[:], oh_d[r, :, gidx * GC:(gidx + 1) * GC, :])
                lg = sp.tile([128, GC, HEADS], F32, tag="lg")
                msg = ep1.tile([128, GC, H + HEADS], BF16, tag="msg")
                nc.vector.tensor_mul(msg[:, :, 0:H], qig[:], kg_ap)
                nc.vector.tensor_reduce(
                    lg[:], msg[:, :, 0:H].rearrange("p g (hh dd) -> p g hh dd", dd=D),
                    mybir.AxisListType.X, OP.add)
                nc.scalar.activation(msg[:, :, H:H + HEADS], lg[:], AF.Exp)
                nc.vector.tensor_mul(
                    msg[:, :, 0:H].rearrange("p g (hh dd) -> p g hh dd", dd=D),
                    vg_ap.rearrange("p g (hh dd) -> p g hh dd", dd=D),
                    bc32(msg[:, :, H:H + HEADS]))
                for wi in range(GWIN):
                    w = gidx * GWIN + wi
                    pw = psC.tile([128, H + HEADS], F32, tag="pw")
                    for kc in range(KCH):
                        nc.tensor.matmul(
                            pw[:], ohg[:, wi * KCH + kc, :],
                            msg[:, wi * KCH + kc, :],
                            start=(kc == 0), stop=(kc == KCH - 1))
                    rec = sp.tile([128, HEADS], F32, tag="rec")
                    # +1e-30: degree-0 dst slots have sum 0; keep 0*recip = 0
                    nc.vector.tensor_scalar_add(rec[:], pw[:, H:H + HEADS], 1e-30)
                    nc.vector.reciprocal(rec[:], rec[:])
                    if r == 0:
                        nc.vector.tensor_mul(
                            agg1[:, w, :].rearrange("p (hh dd) -> p hh dd", dd=D),
                            pw[:, 0:H].rearrange("p (hh dd) -> p hh dd", dd=D),
                            bc32(rec[:]))
                        continue
                    an = stg.tile([128, H], BF16, tag="an")
                    nc.vector.tensor_mul(
                        an[:].rearrange("p (hh dd) -> p hh dd", dd=D),
                        pw[:, 0:H].rearrange("p (hh dd) -> p hh dd", dd=D),
                        bc32(rec[:]))
                    if r == 2:
                        nc.vector.tensor_add(an[:], an[:], agg1[:, w, :])
                    gt = stg.tile([128, H], BF16, tag="gelu")
                    nc.scalar.activation(gt[:], an[:], AF.Gelu)
                    gT = stg.tile([128, 2, 128], BF16, tag="gT")
                    transpose2(gT, gt, w + 1)
                    po = psD.tile([128, H], F32, tag="pp")
                    for kt in range(2):
                        nc.tensor.matmul(po[:], gT[:, kt, :], wa_t[:, kt, :],
                                         start=(kt == 0), stop=(kt == 1))
                    if ba_t is not None:
                        nc.vector.tensor_add(po[:], po[:], ba_t[:])
                    # h_pre = o + h (in place), s1 = row sums
                    nc.vector.scalar_tensor_tensor(
                        h[:, dt, w, :], po[:], 1.0, h[:, dt, w, :],
                        OP.mult, OP.add, accum_out=s1[:, dt, w:w + 1])
                    nc.scalar.activation(sqs[:], h[:, dt, w, :], AF.Square,
                                         accum_out=s2[:, dt, w:w + 1])

            def edge_pair(l, rA, rB):
                ctxA = edge_setup(l, rA)
                ctxB = edge_setup(l, rB)
                for gidx in range(NGRP):
                    edge_group(l, rA, gidx, ctxA, 0)
                for gidx in range(NGRP):
                    edge_group(l, rB, gidx, ctxB, 1)

            def finish_type(l, t):
                mu = sp.tile([128, NT], F32, tag="mu")
                inv = sp.tile([128, NT], F32, tag="inv")
                nmi = sp.tile([128, NT], F32, tag="nmi")
                nc.vector.tensor_scalar_mul(mu[:], s1[:, t, :], 1.0 / H)
                nc.vector.tensor_scalar_mul(inv[:], s2[:, t, :], 1.0 / H)  # mean sq
                musq = sp.tile([128, NT], F32, tag="musq")
                nc.vector.tensor_mul(musq[:], mu[:], mu[:])
                nc.vector.scalar_tensor_tensor(
                    inv[:], inv[:], float(eps_eff[l][t]), musq[:],
                    OP.add, OP.subtract)              # var + eps
                nc.scalar.activation(inv[:], inv[:], AF.Sqrt)
                nc.vector.reciprocal(inv[:], inv[:])
                nc.vector.scalar_tensor_tensor(
                    nmi[:], mu[:], -1.0, inv[:], OP.mult, OP.mult)
                if ub["lng"] or ub["lnb"]:
                    lng_t = load_bias(bias_d["lng_b"][l, t])
                    lnb_t = load_bias(bias_d["lnb_b"][l, t])
                    for w in range(NT):
                        nc.scalar.activation(
                            h[:, t, w, :], h[:, t, w, :], AF.Identity,
                            bias=nmi[:, w:w + 1], scale=inv[:, w:w + 1])
                        nc.vector.tensor_mul(h[:, t, w, :], h[:, t, w, :], lng_t[:])
                        nc.vector.tensor_add(h[:, t, w, :], h[:, t, w, :], lnb_t[:])
                        nc.scalar.activation(h[:, t, w, :], h[:, t, w, :], AF.Relu)
                else:
                    for w in range(NT):
                        nc.scalar.activation(
                            h[:, t, w, :], h[:, t, w, :], AF.Relu,
                            bias=nmi[:, w:w + 1], scale=inv[:, w:w + 1])

            # ---- layer 0: input proj per type, kv projections + AllGathers
            # interleaved so AG0 launches as early as possible; q_r0 (needed
            # first by the edge phase) is projected before AG1's trigger can
            # stall the tensor queue. ----
            input_proj(0)
            hT0 = transposes(0)
            kv_proj_ag(0, 0, hT0)
            input_proj(1)
            hT1 = transposes(1)
            q_proj(0, 0, hT1)
            kv_proj_ag(0, 1, hT1)
            q_proj(0, 1, hT0)
            input_proj(2)
            hT2 = transposes(2)
            q_proj(0, 2, hT1)
            q_proj(0, 3, hT2)

            edge_pair(0, 0, 1)
            finish_type(0, 0)
            hT0b = transposes(0)
            kv_proj_ag(1, 0, hT0b)
            q_proj(1, 1, hT0b)
            edge_pair(0, 2, 3)
            finish_type(0, 1)
            finish_type(0, 2)
            hT1b = transposes(1)
            kv_proj_ag(1, 1, hT1b)
            q_proj(1, 0, hT1b)
            q_proj(1, 2, hT1b)
            hT2b = transposes(2)
            q_proj(1, 3, hT2b)

            edge_pair(1, 0, 1)
            finish_type(1, 0)
            edge_pair(1, 2, 3)
            finish_type(1, 1)
            finish_type(1, 2)

            # ---- output projection ----
            wo = load_w(wout_d)
            bo = load_bias(bias_d["bout_b"]) if ub["bout"] else None
            for t in range(T):
                hTo = transposes(t)
                for nt in range(NT):
                    ps = psD.tile([128, OUT], F32, tag="pp")
                    for kt in range(2):
                        nc.tensor.matmul(ps[:], hTo[:, kt, nt, :], wo[:, kt, :OUT],
                                         start=(kt == 0), stop=(kt == 1))
                    st_ = stg.tile([128, OUT], F32, tag="yout")
                    if bo is not None:
                        nc.vector.tensor_add(st_[:], ps[:], bo[:, :OUT])
                    else:
                        nc.scalar.copy(st_[:], ps[:])
                    nc.sync.dma_start(y_d[t, nt * 128:(nt + 1) * 128, :], st_[:])
    nc.compile()
    return nc


def kernel(**inputs):
    shared, per_core, meta = _preprocess(inputs)
    shapes = {k: list(v.shape) for k, v in {**shared, **per_core[0]}.items()}
    nc = bacc.Bacc("TRN2", target_bir_lowering=False, debug=False, num_devices=NC,
                   num_swdge_queues=2)
    nc = _build(nc, meta, shapes)
    in_maps = [{**shared, **per_core[c]} for c in range(NC)]
    res = run_bass_kernel_spmd(nc, in_maps, core_ids=list(range(NC)))
    core_of, slot_of = meta["core_of"], meta["slot_of"]
    y = np.empty((T, N, OUT), np.float32)
    for c in range(NC):
        yc = np.asarray(res.results[c]["y"], np.float32)
        for t in range(T):
            idx = np.where(core_of[t] == c)[0]
            y[t, idx] = yc[t, slot_of[t][idx]]
    return y


if __name__ == "__main__":
    import reference
    inputs = {k: np.asarray(v) for k, v in reference.setup_inputs().items()}
    out = kernel(**inputs)
    exp = np.asarray(reference.reference(**inputs))
    err = np.abs(out - exp).max() / np.abs(exp).max()
    print("Relative error:", err)
